# revision 4
# baseline (speedup 1.0000x reference)
"""OIM loss forward on 8 Trainium2 NeuronCores (Bass/Tile, SPMD data-parallel).

loss = mean over valid rois of [ logsumexp_j(30*x_i.w_j) - 30*x_i.w_{label_i} ]
with W = concat(lut, cq) [10532, 256], x [4096, 256], label [4096] (5554 = ignore).

Sharding: pure data-parallel over rois — 512 rois per core, W replicated.
Per core on device:
  - PE: bf16 GEMM x_shard @ W.T into PSUM, tiled [128 rois x <=2048 classes]
  - ACT: fused exp(30*logit) + row-sum via accum_out (one pass over PSUM)
  - GPSIMD dma_gather: fetch W[label_i] rows (f32) from HBM
  - DVE tensor_tensor_reduce: label logit dot, then mask + reduce
  - tiny f32 ones-matmul: partition reduction -> per-core [sum_nll, n_valid]
Host: sums the 8 partial pairs, divides.
"""
import sys
import numpy as np

sys.path.insert(0, "/opt/trn_rl_repo")

import ml_dtypes

import concourse.bacc as bacc
import concourse.bass as bass
import concourse.tile as tile
from concourse import bass_utils
from concourse.bass import mybir

F = 256          # feature dim
C = 10532        # num classes = 5532 lut + 5000 cq
N = 4096         # total rois
NCORES = 8
NSH = N // NCORES    # 512 rois per core
RT = NSH // 128      # 4 roi tiles per core
KC = F // 128        # 2 contraction chunks
GW = 2048            # class-group width (4 PSUM banks)
NG = (C + GW - 1) // GW  # 6 groups (last = 292)
IGNORE = 5554
SCALAR = 30.0

_cache = {}


def _build():
    if "nc" in _cache:
        return _cache["nc"]
    nc = bacc.Bacc("TRN2", target_bir_lowering=False, debug=False,
                   num_devices=NCORES)
    dt = mybir.dt

    xT_d = nc.dram_tensor("xT", [128, KC, NSH], dt.bfloat16, kind="ExternalInput").ap()
    wT_d = nc.dram_tensor("wT", [128, KC, C], dt.bfloat16, kind="ExternalInput").ap()
    xr_d = nc.dram_tensor("xr", [128, RT, F], dt.float32, kind="ExternalInput").ap()
    wr_d = nc.dram_tensor("wr", [C, F], dt.float32, kind="ExternalInput").ap()
    idx_d = nc.dram_tensor("idx", [128, NSH // 16], dt.int16, kind="ExternalInput").ap()
    msk_d = nc.dram_tensor("msk", [128, RT], dt.float32, kind="ExternalInput").ap()
    out_d = nc.dram_tensor("partial", [2], dt.float32, kind="ExternalOutput").ap()

    with tile.TileContext(nc) as tc:
        with (
            tc.tile_pool(name="wpool", bufs=2) as wpool,
            tc.tile_pool(name="xpool", bufs=1) as xpool,
            tc.tile_pool(name="spool", bufs=2) as spool,
            tc.tile_pool(name="small", bufs=1) as small,
            tc.tile_pool(name="psum", bufs=2, space="PSUM") as psum,
        ):
            # --- static loads -------------------------------------------------
            xT = xpool.tile([128, KC, NSH], dt.bfloat16)
            nc.sync.dma_start(xT[:], xT_d[:])
            xr = xpool.tile([128, RT, F], dt.float32)
            nc.sync.dma_start(xr[:], xr_d[:])
            idx = small.tile([128, NSH // 16], dt.int16)
            nc.sync.dma_start(idx[:], idx_d[:])
            msk = small.tile([128, RT], dt.float32)
            nc.sync.dma_start(msk[:], msk_d[:])

            # --- label-row gather + dots -------------------------------------
            wlab = xpool.tile([128, RT, F], dt.float32)
            nc.gpsimd.dma_gather(wlab[:], wr_d[:], idx[:], NSH, NSH, F)
            dot = small.tile([128, RT], dt.float32)  # x_i . w_label_i
            dump = xpool.tile([128, F], dt.float32)
            for r in range(RT):
                nc.vector.tensor_mul(dump[:], xr[:, r, :], wlab[:, r, :])
                nc.vector.tensor_reduce(dot[:, r:r + 1], dump[:],
                                        mybir.AxisListType.X, mybir.AluOpType.add)
            ll = small.tile([128, RT], dt.float32)   # 30 * x_i . w_label_i
            nc.vector.tensor_scalar_mul(ll[:], dot[:], SCALAR)

            # --- main GEMM + fused exp/rowsum --------------------------------
            # parts[r][:, g] = sum_j in group g of exp(30 * x_i . w_j)
            parts = [small.tile([128, NG], dt.float32, name=f"parts{r}",
                                tag=f"parts{r}") for r in range(RT)]
            for g in range(NG):
                w0 = g * GW
                gw = min(GW, C - w0)
                wg = wpool.tile([128, KC, gw], dt.bfloat16, tag="wg")
                nc.sync.dma_start(wg[:], wT_d[:, :, w0:w0 + gw])
                for r in range(RT):
                    ps = psum.tile([128, gw], dt.float32, tag="ps")
                    for s in range(0, gw, 512):
                        sw = min(512, gw - s)
                        for k in range(KC):
                            nc.tensor.matmul(
                                ps[:, s:s + sw],
                                xT[:, k, r * 128:(r + 1) * 128],
                                wg[:, k, s:s + sw],
                                start=(k == 0), stop=(k == KC - 1),
                            )
                    scr = spool.tile([128, gw], dt.bfloat16, tag="scr")
                    nc.scalar.activation(
                        scr[:], ps[:], mybir.ActivationFunctionType.Exp,
                        scale=SCALAR, accum_out=parts[r][:, g:g + 1],
                    )

            # --- logsumexp, nll, masked partial sums -------------------------
            se = small.tile([128, RT], dt.float32)
            for r in range(RT):
                nc.vector.tensor_reduce(se[:, r:r + 1], parts[r][:],
                                        mybir.AxisListType.X, mybir.AluOpType.add)
            lse = small.tile([128, RT], dt.float32)
            nc.scalar.activation(lse[:], se[:], mybir.ActivationFunctionType.Ln)
            nll = small.tile([128, RT], dt.float32)
            nc.vector.tensor_sub(nll[:], lse[:], ll[:])
            nllm = small.tile([128, RT], dt.float32)
            nc.vector.tensor_mul(nllm[:], nll[:], msk[:])
            stats = small.tile([128, 2], dt.float32)
            nc.vector.tensor_reduce(stats[:, 0:1], nllm[:],
                                    mybir.AxisListType.X, mybir.AluOpType.add)
            nc.vector.tensor_reduce(stats[:, 1:2], msk[:],
                                    mybir.AxisListType.X, mybir.AluOpType.add)

            # --- partition reduction via ones-matmul -------------------------
            ones = small.tile([128, 1], dt.float32)
            nc.vector.memset(ones[:], 1.0)
            red = psum.tile([2, 1], dt.float32, tag="ps")
            nc.tensor.matmul(red[:], stats[:], ones[:])
            outsb = small.tile([2, 1], dt.float32)
            nc.vector.tensor_copy(outsb[:], red[:])
            nc.sync.dma_start(out_d[:], outsb[:])

    nc.compile()
    _cache["nc"] = nc
    return nc


def _marshal(inputs, label, lut, cq):
    """Host-side shard + layout marshaling. Returns in_maps for 8 cores."""
    x = np.ascontiguousarray(np.asarray(inputs, np.float32))       # [N, F]
    lab = np.asarray(label).astype(np.int64)                        # [N]
    W = np.concatenate([np.asarray(lut, np.float32),
                        np.asarray(cq, np.float32)], axis=0)        # [C, F]

    xb = x.astype(ml_dtypes.bfloat16)
    Wb = W.astype(ml_dtypes.bfloat16)
    # wT[p, k, j] = W[j, k*128+p]
    wT = np.ascontiguousarray(Wb.T.reshape(KC, 128, C).transpose(1, 0, 2))
    wr = np.ascontiguousarray(W)

    in_maps = []
    for c in range(NCORES):
        xs = x[c * NSH:(c + 1) * NSH]          # [512, F]
        xsb = xb[c * NSH:(c + 1) * NSH]
        ls = lab[c * NSH:(c + 1) * NSH]
        # xT[p, k, i] = xs[i, k*128+p]
        xT = np.ascontiguousarray(xsb.T.reshape(KC, 128, NSH).transpose(1, 0, 2))
        # xr[p, r, :] = xs[r*128+p, :]
        xr = np.ascontiguousarray(xs.reshape(RT, 128, F).transpose(1, 0, 2))
        valid = ls != IGNORE
        safe = np.where(valid, ls, 0).astype(np.int16)
        # wrapped: flat i -> [i%16, i//16], replicated to all 8 Q7-core groups
        idx = np.tile(safe.reshape(NSH // 16, 16).T, (8, 1)).astype(np.int16)
        msk = np.ascontiguousarray(
            valid.astype(np.float32).reshape(RT, 128).T)  # [128, RT]
        in_maps.append({"xT": xT, "wT": wT, "xr": xr, "wr": wr,
                        "idx": idx, "msk": msk})
    return in_maps


def kernel(inputs, label, lut, cq, header):
    nc = _build()
    in_maps = _marshal(inputs, label, lut, cq)
    res = bass_utils.run_bass_kernel_spmd(nc, in_maps, list(range(NCORES)))
    s = 0.0
    v = 0.0
    for r in res.results:
        p = np.asarray(r["partial"], np.float64).reshape(-1)
        s += p[0]
        v += p[1]
    return np.float32(s / max(v, 1.0))
